# revision 1
# baseline (speedup 1.0000x reference)
"""Multi-head attention (B=4, S=2048, D=1024, H=16) on 8 trn2 NeuronCores.

Sharding: core = (batch b, head-group g) with b = core//2, g = core%2.
Each core handles one batch and 8 heads (512 of the 1024 d_model dims):
  - host pre-transposes query/key/value[b] -> [1024, 2048] so the device
    never transposes activations (and pre-casts to the matmul dtype)
  - device computes Q^T, K^T (head dims on partitions) and V (natural),
    attention with *transposed* scores S^T = K_h @ Q_h^T so softmax's
    denominator comes out of the PV matmul via a ones-column appended to V
  - output projection vs Wo[g*512:(g+1)*512, :] gives a partial [2048,1024]
  - host sums the two group partials per batch and adds bv@Wo + bo
Matmul operand dtype is MM_DT (bf16 default: full-rate PE streaming + FWL;
f32r fallback: fp22 multiplies at half stream rate). PSUM accumulation and
the softmax normalization chain stay fp32.
"""

import os
import numpy as np
from contextlib import ExitStack

B = 4
S = 2048
D = 1024
H = 16
DK = 64
NCORES = 8
GH = 8          # heads per core (group)
GD = GH * DK    # 512 head dims per core
NCH = GD // 128  # 4 chunks of 128 output dims
KT = S // 128    # 16 key tiles
QC = 1024        # q chunk width for attention
NQC = S // QC    # 2
SC = 512         # s chunk width for projections
NSC = S // SC    # 8
DMT = D // 128   # 8 d_model tiles

MM_DT = os.environ.get("MM_DT", "bf16")  # "bf16" | "f32r"

_CACHE = {}


def _np_mm_dtype():
    if MM_DT == "bf16":
        import ml_dtypes
        return ml_dtypes.bfloat16
    return np.float32


def _build_program():
    import concourse.mybir as mybir
    import concourse.tile as tile
    from concourse import bacc

    f32 = mybir.dt.float32
    dmm = mybir.dt.bfloat16 if MM_DT == "bf16" else mybir.dt.float32r

    nc = bacc.Bacc("TRN2", target_bir_lowering=False, debug=False,
                   num_devices=NCORES)

    xqT = nc.dram_tensor("xqT", [D, S], dmm, kind="ExternalInput").ap()
    xkT = nc.dram_tensor("xkT", [D, S], dmm, kind="ExternalInput").ap()
    xvT = nc.dram_tensor("xvT", [D, S], dmm, kind="ExternalInput").ap()
    wq = nc.dram_tensor("wq", [D, GD], dmm, kind="ExternalInput").ap()
    wk = nc.dram_tensor("wk", [D, GD], dmm, kind="ExternalInput").ap()
    wv = nc.dram_tensor("wv", [D, GD], dmm, kind="ExternalInput").ap()
    wo = nc.dram_tensor("wo", [GD, D], dmm, kind="ExternalInput").ap()
    bq = nc.dram_tensor("bq", [GD], f32, kind="ExternalInput").ap()
    bk = nc.dram_tensor("bk", [GD], f32, kind="ExternalInput").ap()
    out = nc.dram_tensor("out", [S, D], f32, kind="ExternalOutput").ap()

    dbg = os.environ.get("DEBUG_DUMPS", "0") == "1"
    if dbg:
        d_qt = nc.dram_tensor("d_qt", [128, S], dmm, kind="ExternalOutput").ap()
        d_kt = nc.dram_tensor("d_kt", [128, S], dmm, kind="ExternalOutput").ap()
        d_v = nc.dram_tensor("d_v", [128, GH * 65], dmm,
                             kind="ExternalOutput").ap()
        d_pt = nc.dram_tensor("d_pt", [128, QC], dmm,
                              kind="ExternalOutput").ap()
        d_pv = nc.dram_tensor("d_pv", [65, QC], f32, kind="ExternalOutput").ap()
        d_zr = nc.dram_tensor("d_zr", [1, QC], f32, kind="ExternalOutput").ap()
        d_rb = nc.dram_tensor("d_rb", [DK, QC], f32, kind="ExternalOutput").ap()
        d_ot = nc.dram_tensor("d_ot", [128, S], dmm, kind="ExternalOutput").ap()

    Exp = mybir.ActivationFunctionType.Exp

    with tile.TileContext(nc) as tc, ExitStack() as ctx:
        # ---- pools (slots are statically reserved per tag) ----
        p_qt = ctx.enter_context(tc.tile_pool(name="qt", bufs=GH))
        p_kt = ctx.enter_context(tc.tile_pool(name="kt", bufs=GH))
        p_v = ctx.enter_context(tc.tile_pool(name="v", bufs=KT))
        p_ot = ctx.enter_context(tc.tile_pool(name="ot", bufs=NCH))
        p_wvo = ctx.enter_context(tc.tile_pool(name="wvo", bufs=1))
        p_wc = ctx.enter_context(tc.tile_pool(name="wc", bufs=1))
        p_bias = ctx.enter_context(tc.tile_pool(name="bias", bufs=1))
        p_xs = ctx.enter_context(tc.tile_pool(name="xs", bufs=3))
        p_pt = ctx.enter_context(tc.tile_pool(name="pt", bufs=8))
        p_zr = ctx.enter_context(tc.tile_pool(name="zr", bufs=2))
        p_rb = ctx.enter_context(tc.tile_pool(name="rb", bufs=2))
        p_st = ctx.enter_context(tc.tile_pool(name="st", bufs=2))
        p_ov = ctx.enter_context(tc.tile_pool(name="ov", bufs=3))
        # PSUM: 2-bank slots x 2 bufs x 2 pools = all 8 banks
        p_ps = ctx.enter_context(tc.tile_pool(name="ps", bufs=3, space="PSUM"))
        p_pv = ctx.enter_context(tc.tile_pool(name="pv", bufs=1, space="PSUM"))

        # ---- biases + ones ----
        bq_sb = p_bias.tile([128, NCH], f32, tag="bq")
        nc.sync.dma_start(out=bq_sb[:], in_=bq.rearrange("(a p) -> p a", p=128))
        bk_sb = p_bias.tile([128, NCH], f32, tag="bk")
        nc.sync.dma_start(out=bk_sb[:], in_=bk.rearrange("(a p) -> p a", p=128))
        ones_sb = p_bias.tile([128, 1], f32, tag="ones")
        nc.vector.memset(ones_sb[:], 1.0)

        # ---- V projection: V_sb[st] = [128 s, GH, 65] (col 64 = ones) ----
        v_sb = []

        vstate = {}

        def v_filler(st):
            def emit():
                emit_v_st(st)
            return emit

        def emit_v_proj(first=0):
          wv_sb = p_wvo.tile([128, DMT, GD], dmm, tag="wvo", name="wv_sb")
          nc.scalar.dma_start(out=wv_sb[:],
                              in_=wv.rearrange("(a p) d -> p a d", p=128))
          vstate["wv"] = wv_sb
          for st in range(first):
              emit_v_st(st)

        def emit_v_st(st):
          wv_sb = vstate["wv"]
          if True:
              if st % 4 == 0:
                  xv_t = p_xs.tile([128, DMT, 512], dmm, tag="xs",
                                   name=f"xv{st}")
                  nc.scalar.dma_start(
                      out=xv_t[:],
                      in_=xvT[:, st * 128:st * 128 + 512].rearrange(
                          "(a p) s -> p a s", p=128),
                  )
                  vstate["xv"] = xv_t
              xv_t = vstate["xv"]
              sub = (st % 4) * 128
              ps = p_ps.tile([128, 1024], f32, tag="ps", name=f"psv{st}")
              for a in range(DMT):
                  nc.tensor.matmul(
                      out=ps[:, 0:GD],
                      lhsT=xv_t[:, a, sub:sub + 128],
                      rhs=wv_sb[:, a, :],
                      start=(a == 0), stop=(a == DMT - 1),
                  )
              vt = p_v.tile([128, GH, 65], dmm, tag="v", name=f"v{st}")
              nc.vector.tensor_copy(
                  out=vt[:, :, 0:DK],
                  in_=ps[:, 0:GD].rearrange("p (h d) -> p h d", h=GH),
              )
              nc.vector.tensor_copy(
                  out=vt[:, :, DK:65],
                  in_=ones_sb.unsqueeze(1).broadcast_to([128, GH, 1]))
              v_sb.append(vt)

        qt_sb = [None] * GH
        kt_sb = [None] * GH
        ot_sb = [None] * NCH

        wq_sb = p_wc.tile([128, DMT, GD], dmm, tag="wqc", name="wq_sb")
        nc.scalar.dma_start(out=wq_sb[:],
                          in_=wq.rearrange("(a p) d -> p a d", p=128))
        wk_sb = p_wc.tile([128, DMT, GD], dmm, tag="wkc", name="wk_sb")
        nc.scalar.dma_start(out=wk_sb[:],
                          in_=wk.rearrange("(a p) d -> p a d", p=128))

        def proj_fillers(c):
            """Emission groups computing per-head Q^T/K^T for heads 2c,2c+1.
            Each head tile [128, S] holds its 64 dims twice (rows 0-63 and
            64-127) so consecutive kt score matmuls alternate PE row groups
            and run concurrently."""
            for hh in range(2):
                hg = 2 * c + hh
                qt_sb[hg] = p_qt.tile([128, S], dmm, tag="qt", name=f"qt{hg}")
                kt_sb[hg] = p_kt.tile([128, S], dmm, tag="kt", name=f"kt{hg}")

            def group(src, wsb, bsb, dsts, nm, sc):
                def emit():
                    xs = p_xs.tile([128, DMT, SC], dmm, tag="xs",
                                   name=f"xs{nm}{c}_{sc}")
                    nc.sync.dma_start(
                        out=xs[:],
                        in_=src[:, sc * SC:(sc + 1) * SC].rearrange(
                            "(a p) s -> p a s", p=128),
                    )
                    ps = p_ps.tile([128, 1024], f32, tag="ps",
                                   name=f"psp{nm}{c}_{sc}")
                    for a in range(DMT):
                        nc.tensor.matmul(
                            out=ps[:, 0:SC],
                            lhsT=wsb[:, a, c * 128:(c + 1) * 128],
                            rhs=xs[:, a, :],
                            start=(a == 0), stop=(a == DMT - 1),
                        )
                    s0, s1 = sc * SC, (sc + 1) * SC
                    # head 2c native rows 0-63; head 2c+1 native rows 64-127
                    nc.vector.tensor_scalar_add(
                        out=dsts[0][0:DK, s0:s1], in0=ps[0:DK, 0:SC],
                        scalar1=bsb[0:DK, c:c + 1])
                    nc.vector.tensor_scalar_add(
                        out=dsts[1][DK:128, s0:s1], in0=ps[DK:128, 0:SC],
                        scalar1=bsb[DK:128, c:c + 1])
                    # duplicate this slice into the other half right away
                    # (SBUF->SBUF DMA) so scores kt for this s-range unblock
                    nc.sync.dma_start(out=dsts[0][DK:128, s0:s1],
                                      in_=dsts[0][0:DK, s0:s1])
                    nc.sync.dma_start(out=dsts[1][0:DK, s0:s1],
                                      in_=dsts[1][DK:128, s0:s1])
                return emit

            qd = [qt_sb[2 * c], qt_sb[2 * c + 1]]
            kd = [kt_sb[2 * c], kt_sb[2 * c + 1]]
            q = lambda sc: group(xqT, wq_sb, bq_sb, qd, "q", sc)
            k = lambda sc: group(xkT, wk_sb, bk_sb, kd, "k", sc)
            # K sc0 + Q sc0/1 first: pair c's scores kt=0 needs them
            return [k(0), q(0), q(1), k(1), k(2), q(2), k(3), q(3)]

        def attention_pair(c, fillers=(), pace=4, slow_fillers=(),
                           qc1_fillers=()):
            """Heads 2c, 2c+1 -> normalized O^T chunk c [128 dout, S].
            fillers: emission callbacks interleaved into the kt loop so
            next-chunk projections share PE/PSUM without starving ACT."""
            fillers = list(fillers)
            slow_fillers = list(slow_fillers)
            qc1_fillers = list(qc1_fillers)
            ot_sb[c] = p_ot.tile([128, S], dmm, tag="ot", name=f"ot{c}")
            for qc in range(NQC):
                ovts = [None, None]
                for hh in range(2):
                    hg = 2 * c + hh
                    pv_ps = p_pv.tile([65, QC], f32, tag="pv",
                                      name=f"pv{c}_{qc}_{hh}")
                    for kt_i in range(KT):
                        if fillers and (pace == 1 or kt_i % pace == 2):
                            fillers.pop(0)()
                        elif slow_fillers and kt_i % 4 == 2:
                            slow_fillers.pop(0)()
                        elif qc == 1 and qc1_fillers and kt_i % 4 == 2:
                            qc1_fillers.pop(0)()
                        rg = DK * (kt_i % 2)
                        ps = p_ps.tile([128, QC], f32, tag="ps",
                                       name=f"pss{c}_{qc}_{kt_i}_{hh}")
                        for half in range(QC // 512):
                            q0 = qc * QC + half * 512
                            nc.tensor.matmul(
                                out=ps[:, half * 512:(half + 1) * 512],
                                lhsT=kt_sb[hg][rg:rg + DK,
                                               kt_i * 128:(kt_i + 1) * 128],
                                rhs=qt_sb[hg][rg:rg + DK, q0:q0 + 512],
                                start=True, stop=True,
                            )
                        pt = p_pt.tile([128, QC], dmm, tag="pt",
                                       name=f"pt{c}_{qc}_{kt_i}_{hh}")
                        nc.scalar.activation(pt[:], ps[:], Exp,
                                             bias=0.0, scale=0.125)
                        if dbg and c == 0 and qc == 0 and kt_i == 0 and hh == 0:
                            nc.sync.dma_start(out=d_pt[:], in_=pt[:])
                        for half in range(QC // 512):
                            nc.tensor.matmul(
                                out=pv_ps[:, half * 512:(half + 1) * 512],
                                lhsT=v_sb[kt_i][:, hg, :],
                                rhs=pt[:, half * 512:(half + 1) * 512],
                                start=(kt_i == 0), stop=(kt_i == KT - 1),
                            )
                    # evict PV psum right away to release its bank pair
                    ovt = p_ov.tile([65, QC], f32, tag="ov",
                                    name=f"ov{c}_{qc}_{hh}")
                    nc.vector.tensor_copy(out=ovt[:], in_=pv_ps[:])
                    ovts[hh] = ovt
                # normalize off the critical path:
                # O^T = PV[0:64] * broadcast(1 / PV[64])
                for hh in range(2):
                    ovt = ovts[hh]
                    # reciprocal of Z across 64 lanes: scatter -> recip -> gather
                    zs = p_zr.tile([DK, QC // DK], f32, tag="zs",
                                   name=f"zs{c}_{qc}_{hh}")
                    nc.sync.dma_start(out=zs[:], in_=ovt[DK:DK + 1, :])
                    nc.vector.reciprocal(out=zs[:], in_=zs[:])
                    zr = p_zr.tile([1, QC], f32, tag="zr",
                                   name=f"zr{c}_{qc}_{hh}")
                    nc.sync.dma_start(out=zr[:], in_=zs[:])
                    rb = p_rb.tile([DK, QC], f32, tag="rb",
                                   name=f"rb{c}_{qc}_{hh}")
                    nc.gpsimd.partition_broadcast(rb[:], zr[:], channels=DK)
                    if dbg and c == 0 and qc == 0 and hh == 0:
                        nc.sync.dma_start(out=d_pv[:], in_=ovt[:])
                        nc.sync.dma_start(out=d_zr[:], in_=zr[:])
                        nc.sync.dma_start(out=d_rb[:], in_=rb[:])
                    if hh == 0:
                        nc.vector.tensor_mul(
                            out=ot_sb[c][0:DK, qc * QC:(qc + 1) * QC],
                            in0=ovt[0:DK, :], in1=rb[:])
                    else:
                        tmp = p_rb.tile([DK, QC], dmm, tag="rb",
                                        name=f"tmp{c}_{qc}")
                        nc.vector.tensor_mul(out=tmp[:], in0=ovt[0:DK, :],
                                             in1=rb[:])
                        nc.sync.dma_start(
                            out=ot_sb[c][DK:128, qc * QC:(qc + 1) * QC],
                            in_=tmp[:])


        # ---- output projection ----
        def emit_final(qts):
          for qt_i in qts:
              ps = p_ps.tile([128, 1024], f32, tag="ps", name=f"pso{qt_i}")
              for c in range(NCH):
                  for half in range(2):
                      nc.tensor.matmul(
                          out=ps[:, half * 512:(half + 1) * 512],
                          lhsT=ot_sb[c][:, qt_i * 128:(qt_i + 1) * 128],
                          rhs=wo_sb[:, c, half * 512:(half + 1) * 512],
                          start=(c == 0), stop=(c == NCH - 1),
                      )
              st = p_st.tile([128, D], f32, tag="st", name=f"st{qt_i}")
              nc.vector.tensor_copy(out=st[:], in_=ps[:])
              nc.sync.dma_start(out=out[qt_i * 128:(qt_i + 1) * 128, :], in_=st[:])

        # ---- emit: QK chunk 0 first so attention starts ASAP; V proj
        # streams in behind it; later chunk projections fill PE gaps ----
        wo_sb = p_wvo.tile([128, NCH, D], dmm, tag="wo", name="wo_sb")
        g0 = proj_fillers(0)
        for g in g0[:3]:        # k0, q0, q1 -> first scores ready ASAP
            g()
        emit_v_proj(first=4)    # v st0-3 for pair0's first PV steps
        nc.scalar.dma_start(out=wo_sb[:],
                          in_=wo.rearrange("(a p) n -> p a n", p=128))
        # everything else streams into pair0's PE gaps, one group per kt,
        # ordered so each V tile and K^T slice lands just before use
        k1, k2, q2, k3, q3 = g0[3], g0[4], g0[5], g0[6], g0[7]
        f0 = [v_filler(4), k1, v_filler(5), k2, v_filler(6), v_filler(7),
              k3, v_filler(8), v_filler(9), v_filler(10), v_filler(11),
              v_filler(12), v_filler(13), v_filler(14), v_filler(15)]
        attention_pair(0, fillers=f0, pace=1,
                       slow_fillers=[q2, q3] + proj_fillers(1))
        attention_pair(1, fillers=proj_fillers(2))
        attention_pair(2, fillers=proj_fillers(3))
        fin = [(lambda q: (lambda: emit_final([q])))(q) for q in range(8)]
        attention_pair(3, qc1_fillers=fin)
        emit_final(range(8, KT))

        if dbg:
            nc.sync.dma_start(out=d_qt[:], in_=qt_sb[0][:])
            nc.sync.dma_start(out=d_kt[:], in_=kt_sb[0][:])
            nc.sync.dma_start(out=d_v[:],
                              in_=v_sb[0][:].rearrange("p a b -> p (a b)"))
            nc.sync.dma_start(out=d_ot[:], in_=ot_sb[0][:])

    nc.compile()
    return nc


def get_program():
    if "nc" not in _CACHE:
        _CACHE["nc"] = _build_program()
    return _CACHE["nc"]


def make_in_maps(inputs):
    dt = _np_mm_dtype()
    q = np.asarray(inputs["query"], np.float32)
    k = np.asarray(inputs["key"], np.float32)
    v = np.asarray(inputs["value"], np.float32)
    Wq = np.asarray(inputs["Wq"], np.float32)
    Wk = np.asarray(inputs["Wk"], np.float32)
    Wv = np.asarray(inputs["Wv"], np.float32)
    Wo = np.asarray(inputs["Wo"], np.float32)
    bq = np.asarray(inputs["bq"], np.float32)
    bk = np.asarray(inputs["bk"], np.float32)
    in_maps = []
    for core in range(NCORES):
        b, g = core // 2, core % 2
        sl = slice(g * GD, (g + 1) * GD)
        in_maps.append({
            "xqT": np.ascontiguousarray(q[b].T).astype(dt),
            "xkT": np.ascontiguousarray(k[b].T).astype(dt),
            "xvT": np.ascontiguousarray(v[b].T).astype(dt),
            "wq": np.ascontiguousarray(Wq[:, sl]).astype(dt),
            "wk": np.ascontiguousarray(Wk[:, sl]).astype(dt),
            "wv": np.ascontiguousarray(Wv[:, sl]).astype(dt),
            "wo": np.ascontiguousarray(Wo[sl, :]).astype(dt),
            "bq": np.ascontiguousarray(bq[sl]),
            "bk": np.ascontiguousarray(bk[sl]),
        })
    return in_maps


def combine_outputs(results, inputs):
    Wo = np.asarray(inputs["Wo"], np.float32)
    bv = np.asarray(inputs["bv"], np.float32)
    bo = np.asarray(inputs["bo"], np.float32)
    out = np.empty((B, S, D), np.float32)
    for b in range(B):
        out[b] = results[2 * b]["out"] + results[2 * b + 1]["out"]
    out += bv @ Wo + bo
    return out


def kernel(**inputs):
    from concourse.bass_utils import run_bass_kernel_spmd
    nc = get_program()
    in_maps = make_in_maps(inputs)
    res = run_bass_kernel_spmd(nc, in_maps, list(range(NCORES)))
    return combine_outputs(res.results, inputs)

